# revision 5
# baseline (speedup 1.0000x reference)
"""2D Haar DWT (single level) on Trainium2, 8 NeuronCores, pure data parallel.

Math: with Haar filters + symmetric pad + odd-phase downsample, the DWT
reduces to per-2x2-block butterflies over the input image x:
  ll = 0.5*(x00 + x01 + x10 + x11)   (top-left quadrant of output)
  lh = 0.5*(x00 + x01 - x10 - x11)   (bottom-left)
  hl = 0.5*(x00 - x01 + x10 - x11)   (top-right)
  hh = 0.5*(x00 - x01 - x10 + x11)   (bottom-right)

Layout is chosen for MAX DMA DESCRIPTOR SIZE (measured: 2KB descs ~230-270
B/ns, 8KB descs ~380 B/ns per stream): units of TWO images, partition
(jj*64 + p') holds 8 consecutive rows 8p'..8p'+7 of image jj.
  in-DMA   16KB contiguous per partition (SP HWDGE ring).
  out-DMA  rows 4p'..4p'+3 of [ll|hl] (resp [lh|hh]) = 8KB contiguous
           per partition (ACT HWDGE ring, separate FIFO from in).
Compute (all free-axis, H row-pairs stay within a partition):
  W pass   sums on DVE via strided tensor_tensor f32->bf16 (output-rate 1x;
           beats tensor_reduce which is input-rate 1x), diffs split
           GpSimd (3/4) + DVE (1/4).
  scale    T *= 0.5 in one dense bf16 tensor_scalar (4x mode) -- exact
           (power of two), so no ACTIVATE scale/cast pass exists at all.
  H pass   2 tensor_tensor ops per span (top=sums, bottom=diffs),
           bf16 in -> f32 out directly.
First unit is processed in row-quarters and last unit in halves to start
the out stream early and keep the exposed tail short.
"""

import numpy as np

import concourse.mybir as mybir
from concourse import bacc, tile
from concourse.bass_utils import run_bass_kernel_spmd

N_CORES = 8
BATCH = 64
B_PER = BATCH // N_CORES  # 8 images per core
H = W = 512

_nc_cache = None


def build_bass():
    f32 = mybir.dt.float32
    bf16 = mybir.dt.bfloat16
    nc = bacc.Bacc(
        "TRN2", target_bir_lowering=False, debug=False, num_devices=N_CORES
    )
    inp = nc.dram_tensor("inputs", [B_PER, H, W], f32, kind="ExternalInput").ap()
    out = nc.dram_tensor("out", [B_PER, H, W], f32, kind="ExternalOutput").ap()

    with tile.TileContext(nc) as tc:
        with tc.tile_pool(name="p", bufs=3) as pool:

            def unit(u, in_spans, spans, ogroups):
                i0 = 2 * u
                X = pool.tile([128, 4096], f32, tag="X", bufs=3)
                src = inp[i0 : i0 + 2].rearrange(
                    "jj (p r) w -> (jj p) (r w)", p=64
                )
                for rlo, rhi in in_spans:
                    nc.sync.dma_start(
                        out=X[:, rlo * 512 : rhi * 512],
                        in_=src[:, rlo * 512 : rhi * 512],
                    )
                T = pool.tile([128, 4096], bf16, tag="T", bufs=3)
                Y = pool.tile([128, 4096], f32, tag="Y", bufs=3)
                Xv = X[:].rearrange("p (r j two) -> p r j two", r=8, two=2)
                Tv = T[:].rearrange("p (r u j) -> p r u j", r=8, u=2)
                Tp = T[:].rearrange("p (t r2 uj) -> p t r2 uj", t=4, r2=2)
                Yv = Y[:].rearrange("p (h t uj) -> p h t uj", h=2, t=4)
                with nc.allow_low_precision(reason="bf16 DWT intermediates"):
                    for rlo, rhi in spans:
                        nc.vector.tensor_add(
                            out=Tv[:, rlo:rhi, 0],
                            in0=Xv[:, rlo:rhi, :, 0],
                            in1=Xv[:, rlo:rhi, :, 1],
                        )
                        rsp = max(rlo + (rhi - rlo) * 3 // 4, rlo + 1)
                        nc.gpsimd.tensor_sub(
                            out=Tv[:, rlo:rsp, 1],
                            in0=Xv[:, rlo:rsp, :, 0],
                            in1=Xv[:, rlo:rsp, :, 1],
                        )
                        if rsp < rhi:
                            nc.vector.tensor_sub(
                                out=Tv[:, rsp:rhi, 1],
                                in0=Xv[:, rsp:rhi, :, 0],
                                in1=Xv[:, rsp:rhi, :, 1],
                            )
                        nc.vector.tensor_scalar_mul(
                            T[:, rlo * 512 : rhi * 512],
                            T[:, rlo * 512 : rhi * 512],
                            0.5,
                        )
                        tlo, thi = rlo // 2, rhi // 2
                        nc.vector.tensor_add(
                            out=Yv[:, 0, tlo:thi],
                            in0=Tp[:, tlo:thi, 0],
                            in1=Tp[:, tlo:thi, 1],
                        )
                        nc.vector.tensor_sub(
                            out=Yv[:, 1, tlo:thi],
                            in0=Tp[:, tlo:thi, 0],
                            in1=Tp[:, tlo:thi, 1],
                        )
                for tlo, thi in ogroups:
                    for jj in range(2):
                        ys = Y[jj * 64 : (jj + 1) * 64].rearrange(
                            "p (h t w) -> p h t w", h=2, t=4
                        )
                        for h in range(2):
                            dst = out[i0 + jj][
                                h * 256 : (h + 1) * 256
                            ].rearrange("(p t) w -> p t w", p=64)[:, tlo:thi]
                            nc.scalar.dma_start(
                                out=dst, in_=ys[:, h, tlo:thi]
                            )

            Q = [(0, 2), (2, 4), (4, 6), (6, 8)]
            HV = [(0, 4), (4, 8)]
            FULL = [(0, 8)]
            unit(0, Q, Q, [(0, 2), (2, 4)])
            unit(1, FULL, FULL, [(0, 4)])
            unit(2, FULL, FULL, [(0, 4)])
            unit(3, HV, HV, [(0, 2), (2, 4)])

    nc.compile()
    return nc


def kernel(**inputs):
    global _nc_cache
    x = np.ascontiguousarray(
        np.asarray(inputs["inputs"], dtype=np.float32).reshape(BATCH, H, W)
    )
    if _nc_cache is None:
        _nc_cache = build_bass()
    nc = _nc_cache
    in_maps = [
        {"inputs": x[i * B_PER : (i + 1) * B_PER]} for i in range(N_CORES)
    ]
    res = run_bass_kernel_spmd(nc, in_maps, core_ids=list(range(N_CORES))).results
    out = np.concatenate([res[i]["out"] for i in range(N_CORES)], axis=0)
    return out.reshape(BATCH, H, W, 1)


# revision 6
# speedup vs baseline: 1.0558x; 1.0558x over previous
"""2D Haar DWT (single level) on Trainium2, 8 NeuronCores, pure data parallel.

Math: with Haar filters + symmetric pad + odd-phase downsample, the DWT
reduces to per-2x2-block butterflies over the input image x:
  ll = 0.5*(x00 + x01 + x10 + x11)   (top-left quadrant of output)
  lh = 0.5*(x00 + x01 - x10 - x11)   (bottom-left)
  hl = 0.5*(x00 - x01 + x10 - x11)   (top-right)
  hh = 0.5*(x00 - x01 - x10 + x11)   (bottom-right)

DMA shapes are the measured-fastest ones: partition p holds 4 consecutive
rows 4p..4p+3 of one image -> in-DMA 8KB contiguous descriptors (SP HWDGE
ring, ~380 B/ns); out-DMA per (image, half): partition p holds DRAM rows
{2p, 2p+1} of [ll|hl] (resp [lh|hh]) = 4KB contiguous descriptors (ACT
HWDGE ring, separate FIFO so the two streams overlap).

Compute (everything free-axis, measured per-op costs):
  W sums   DVE strided tensor_tensor f32->bf16 at output rate (~1.1us/img)
           -- replaces tensor_reduce, which is always-1x at INPUT rate.
  W diffs  GpSimd, ONE wide op per image (~1.3us/img; wide ops halve the
           per-op overhead vs narrow slices).
  scale    T *= 0.5 in one dense bf16 tensor_scalar (4x mode, exact since
           0.5 is a power of two) -- kills the 18us ACTIVATE scale+cast.
  H pass   2 tensor_tensor per image, bf16 in -> f32 out; H-bot offloaded
           to GpSimd for half the images to balance DVE.
ACT does nothing but issue out-DMAs; first/last images split for early
out-start / short tail.
"""

import numpy as np

import concourse.mybir as mybir
from concourse import bacc, tile
from concourse.bass_utils import run_bass_kernel_spmd

N_CORES = 8
BATCH = 64
B_PER = BATCH // N_CORES  # 8 images per core
H = W = 512

_nc_cache = None


def build_bass():
    f32 = mybir.dt.float32
    bf16 = mybir.dt.bfloat16
    nc = bacc.Bacc(
        "TRN2", target_bir_lowering=False, debug=False, num_devices=N_CORES
    )
    inp = nc.dram_tensor("inputs", [B_PER, H, W], f32, kind="ExternalInput").ap()
    out = nc.dram_tensor("out", [B_PER, H, W], f32, kind="ExternalOutput").ap()

    with tile.TileContext(nc) as tc:
        with tc.tile_pool(name="p", bufs=3) as pool:
            for i in range(B_PER):
                X = pool.tile([128, 2048], f32, tag="X", bufs=4)
                src = inp[i].rearrange("(p r) w -> p (r w)", p=128)
                if i == 0:  # halve the first fill to start compute sooner
                    nc.sync.dma_start(out=X[:, :1024], in_=src[:, :1024])
                    nc.sync.dma_start(out=X[:, 1024:], in_=src[:, 1024:])
                else:
                    nc.sync.dma_start(out=X[:], in_=src)

                T = pool.tile([128, 2048], bf16, tag="T", bufs=3)
                Y = pool.tile([128, 2048], f32, tag="Y", bufs=3)
                # X[p, r*512+w] = x[4p+r, w];  T[p, r*512+u*256+j]:
                # u=0 col-pair sums, u=1 diffs;  r = 2q + r2.
                Xv = X[:].rearrange("p (r j two) -> p r j two", r=4, two=2)
                Tv = T[:].rearrange("p (r u j) -> p r u j", r=4, u=2)
                Tq = T[:].rearrange("p (q r2 uj) -> p q r2 uj", q=2, r2=2)
                # Y[p, c*1024+q*512+w] = out[c*256+2p+q, w]
                Yv = Y[:].rearrange("p (c q w) -> p c q w", c=2, q=2)
                with nc.allow_low_precision(reason="bf16 DWT intermediates"):
                    spans = [(0, 2), (2, 4)] if i == 0 else [(0, 4)]
                    for rlo, rhi in spans:
                        nc.vector.tensor_add(
                            out=Tv[:, rlo:rhi, 0],
                            in0=Xv[:, rlo:rhi, :, 0],
                            in1=Xv[:, rlo:rhi, :, 1],
                        )
                        nc.gpsimd.tensor_sub(
                            out=Tv[:, rlo:rhi, 1],
                            in0=Xv[:, rlo:rhi, :, 0],
                            in1=Xv[:, rlo:rhi, :, 1],
                        )
                        nc.vector.tensor_scalar_mul(
                            T[:, rlo * 512 : rhi * 512],
                            T[:, rlo * 512 : rhi * 512],
                            0.5,
                        )
                        qlo, qhi = rlo // 2, rhi // 2
                        nc.vector.tensor_add(
                            out=Yv[:, 0, qlo:qhi],
                            in0=Tq[:, qlo:qhi, 0],
                            in1=Tq[:, qlo:qhi, 1],
                        )
                        hbot = nc.gpsimd if i % 2 else nc.vector
                        hbot.tensor_sub(
                            out=Yv[:, 1, qlo:qhi],
                            in0=Tq[:, qlo:qhi, 0],
                            in1=Tq[:, qlo:qhi, 1],
                        )
                for c in range(2):
                    dst = out[i][c * 256 : (c + 1) * 256].rearrange(
                        "(p q) w -> p (q w)", p=128
                    )
                    ysl = Y[:, c * 1024 : (c + 1) * 1024]
                    if i in (0, B_PER - 1):  # early first out / short tail
                        nc.scalar.dma_start(out=dst[:, :512], in_=ysl[:, :512])
                        nc.scalar.dma_start(out=dst[:, 512:], in_=ysl[:, 512:])
                    else:
                        nc.scalar.dma_start(out=dst, in_=ysl)

    nc.compile()
    return nc


def kernel(**inputs):
    global _nc_cache
    x = np.ascontiguousarray(
        np.asarray(inputs["inputs"], dtype=np.float32).reshape(BATCH, H, W)
    )
    if _nc_cache is None:
        _nc_cache = build_bass()
    nc = _nc_cache
    in_maps = [
        {"inputs": x[i * B_PER : (i + 1) * B_PER]} for i in range(N_CORES)
    ]
    res = run_bass_kernel_spmd(nc, in_maps, core_ids=list(range(N_CORES))).results
    out = np.concatenate([res[i]["out"] for i in range(N_CORES)], axis=0)
    return out.reshape(BATCH, H, W, 1)


# revision 7
# speedup vs baseline: 1.3709x; 1.2984x over previous
"""2D Haar DWT (single level) on Trainium2, 8 NeuronCores, pure data parallel.

Math: with Haar filters + symmetric pad + odd-phase downsample, the DWT
reduces to per-2x2-block butterflies over the input image x:
  ll = 0.5*(x00 + x01 + x10 + x11)   (top-left quadrant of output)
  lh = 0.5*(x00 + x01 - x10 - x11)   (bottom-left)
  hl = 0.5*(x00 - x01 + x10 - x11)   (top-right)
  hh = 0.5*(x00 - x01 - x10 + x11)   (bottom-right)

Pipeline of units per core (8 images): [1, 2, 2, 2, 1] image units —
tapered so the first out-DMA starts early (more load/store overlap on
the HBM stream) and the tail chain is short.  In-DMAs on the SP HWDGE ring; out-DMAs on the ACT
ring (separate FIFO rings avoid head-of-line blocking).

Width-pass pair SUMS via one DVE tensor_reduce reading X sequentially
(DVE pays ~3x for strided reads — avoid); width-pass pair DIFFS on
GpSimd with strided reads (software engine, stride-insensitive).  Both
write bf16 T (rel-err budget 2e-2; bf16 keeps the height pass in DVE 2x
mode).  Height pass: wide 2-level-AP bf16 adds/subs on DVE.  ACT
ACTIVATEs apply the 0.5 scale AND cast bf16->f32, then out-DMAs stream
per half.

Per unit: X[128, 2048*n], partition p holds rows 4p..4p+3 per image;
per image Y[p, c*1024 + q*512 + w] = out[c*256 + 2p + q, w].
"""

import numpy as np

import concourse.mybir as mybir
from concourse import bacc, tile
from concourse.bass_utils import run_bass_kernel_spmd

N_CORES = 8
BATCH = 64
B_PER = BATCH // N_CORES  # 8 images per core
H = W = 512

_nc_cache = None


def build_bass():
    f32 = mybir.dt.float32
    bf16 = mybir.dt.bfloat16
    nc = bacc.Bacc(
        "TRN2", target_bir_lowering=False, debug=False, num_devices=N_CORES
    )
    inp = nc.dram_tensor("inputs", [B_PER, H, W], f32, kind="ExternalInput").ap()
    out = nc.dram_tensor("out", [B_PER, H, W], f32, kind="ExternalOutput").ap()

    with tile.TileContext(nc) as tc:
        pool_cm = tc.tile_pool(name="p", bufs=3)
        pool = pool_cm.__enter__()

        def pair_unit(i, n):
            """n consecutive full images starting at image i."""
            F = 2048 * n
            X = pool.tile([128, F], f32, tag="X", bufs=4)
            nc.sync.dma_start(
                out=X[:],
                in_=inp[i : i + n].rearrange("j (p r) w -> p j r w", p=128),
            )
            # width pass: per image j, T[:, j*2048+0:1024] = pair sums
            # (r-blocks of 256), T[:, j*2048+1024:2048] = diffs
            T = pool.tile([128, F], bf16, tag="T")
            with nc.allow_low_precision(reason="bf16 DWT intermediates"):
                nc.vector.tensor_reduce(
                    out=T[:].rearrange("p (j d x) -> p j d x", j=n, d=2)[:, :, 0, :],
                    in_=X[:].rearrange("p (j r k t) -> p (j r) k t", j=n, r=4, t=2),
                    axis=mybir.AxisListType.X,
                    op=mybir.AluOpType.add,
                )
            for j in range(n):
                for r in range(4):
                    o = j * 2048
                    nc.gpsimd.tensor_sub(
                        out=T[:, o + 1024 + r * 256 : o + 1024 + (r + 1) * 256],
                        in0=X[:, o + r * 512 : o + (r + 1) * 512 : 2],
                        in1=X[:, o + r * 512 + 1 : o + (r + 1) * 512 : 2],
                    )
            # height pass (bf16 2x on DVE), unit-wide 2-level ops
            Yb = pool.tile([128, F], bf16, tag="Yb", bufs=4)
            Tv = T[:].rearrange("p (j d q r k) -> p j d q r k", j=n, d=2, q=2, r=2)
            Yv = Yb[:].rearrange("p (j c q h k) -> p j c q h k", j=n, c=2, q=2, h=2)
            nc.vector.tensor_add(
                out=Yv[:, :, 0, :, 0, :], in0=Tv[:, :, 0, :, 0, :], in1=Tv[:, :, 0, :, 1, :]
            )
            nc.vector.tensor_sub(
                out=Yv[:, :, 1, :, 0, :], in0=Tv[:, :, 0, :, 0, :], in1=Tv[:, :, 0, :, 1, :]
            )
            nc.vector.tensor_add(
                out=Yv[:, :, 0, :, 1, :], in0=Tv[:, :, 1, :, 0, :], in1=Tv[:, :, 1, :, 1, :]
            )
            nc.vector.tensor_sub(
                out=Yv[:, :, 1, :, 1, :], in0=Tv[:, :, 1, :, 0, :], in1=Tv[:, :, 1, :, 1, :]
            )
            # fused 0.5 scale + bf16->f32 cast on ACT, then 512 KiB out-DMAs
            Y = pool.tile([128, F], f32, tag="Y", bufs=4)
            for j in range(n):
                for c in range(2):
                    sl = slice(j * 2048 + c * 1024, j * 2048 + (c + 1) * 1024)
                    nc.scalar.mul(Y[:, sl], Yb[:, sl], 0.5)
                    nc.scalar.dma_start(
                        out=out[i + j, c * 256 : (c + 1) * 256].rearrange(
                            "(p q) w -> p q w", q=2
                        ),
                        in_=Y[:, sl],
                    )

        pair_unit(0, 1)
        pair_unit(1, 2)
        pair_unit(3, 2)
        pair_unit(5, 2)
        pair_unit(7, 1)

        pool_cm.__exit__(None, None, None)
    # close TileContext via with-block semantics above

    nc.compile()
    return nc


def kernel(**inputs):
    global _nc_cache
    x = np.ascontiguousarray(
        np.asarray(inputs["inputs"], dtype=np.float32).reshape(BATCH, H, W)
    )
    if _nc_cache is None:
        _nc_cache = build_bass()
    nc = _nc_cache
    in_maps = [
        {"inputs": x[i * B_PER : (i + 1) * B_PER]} for i in range(N_CORES)
    ]
    res = run_bass_kernel_spmd(nc, in_maps, core_ids=list(range(N_CORES))).results
    out = np.concatenate([res[i]["out"] for i in range(N_CORES)], axis=0)
    return out.reshape(BATCH, H, W, 1)

